# revision 31
# baseline (speedup 1.0000x reference)
"""Trainium2 Bass kernel for the DenseSNN problem (4-layer LIF spiking MLP).

Strategy
--------
Data-parallel over batch: B=128 is split into 8 shards of 16, one per
NeuronCore, with weights replicated (no collectives at all).

Per core the time recurrence is restructured layer-at-a-time: layer l's
input spikes for ALL timesteps are known once layer l-1's LIF scan
finishes, so each layer becomes a sequence of batched matmuls over
(t, b) column chunks followed by a sequential 64-step elementwise LIF
scan on the Vector engine, run on the negated membrane m̃ = -mem/th (the
-1/th is folded into weights/bias host-side).

The LIF step is ONE custom DVE instruction (registered at import):

    m̃(t) = beta*m̃(t-1) + c̃(t) + (m̃(t-1) < -1)
    spk(t) = (m̃(t) < -1)                        (flushed 4 steps at a time)

All matmul operands are fp8 e4m3 in DoubleRow perf mode (K=256 per
instruction — 157 TF/s, 2x the bf16 rate). Spikes are exactly
representable in fp8 (0.0/1.0); weights are pre-scaled by 2^12
host-side so their magnitudes sit in e4m3's normal range, and the scale
is divided back out (exact power of two) during the PSUM->SBUF
evacuation on the Scalar engine, which also adds the bias.

Schedule: 16-step column chunks everywhere, strict layer-sequential PE
order (each layer's PE time far exceeds the scan lag, so the PE never
stalls after the initial DMA), weight/x DMAs issued in priority order
across 4 engine queues (x chunk0 + w1 first) with per-slice gating so
the first matmul starts as soon as ~0.75MB has landed. The output layer
writes its full membrane history; spikes are extracted chunk-wise and
accumulated with one tensor_reduce at the end.
"""

import os
import sys

import numpy as np
import ml_dtypes

if "/opt/trn_rl_repo" not in sys.path:
    sys.path.insert(0, "/opt/trn_rl_repo")

T, B, D_IN, D_H, D_OUT = 64, 128, 1024, 2048, 1000
NCORES = 8
BS = B // NCORES           # 16 batch rows per core
COLS = T * BS              # 1024 (t, b) columns
NT = 16                    # timesteps per column chunk
NCH = T // NT              # 4 chunks per layer

WSCALE = 4096.0            # weight pre-scale into e4m3 normal range
XSCALE = 16.0              # x pre-scale

BF16 = ml_dtypes.bfloat16
FP8 = ml_dtypes.float8_e4m3

_COMPILED = {}
_CUSTOM_OPS = None


def _register_custom_ops():
    """Register two fused custom DVE ops (each lowers to a single uop, so
    they run at native 1-instruction DVE throughput):

      LIF_STEP_ANT: out = in0*s0 + in1 + (in0 < s1)
        i.e. m̃(t) = beta*m̃(t-1) + c̃(t) + spike(t-1), with the reset term
        recomputed from the previous membrane.

      ACC_STEP_ANT: out = in0 + (in1 < s1)
        i.e. acc += spike(t) for the output layer's spike counting —
        no spike materialization, no reduction tail.
    """
    global _CUSTOM_OPS
    if _CUSTOM_OPS is not None:
        return _CUSTOM_OPS
    from concourse.dve_spec import Spec, Src0, Src1, C0, C1, lower, _has_src1
    from concourse.dve_uop import DveOpSpec
    from concourse import dve_ops as D

    specs = {
        "LIF_STEP_ANT": Spec(
            body=(Src0 * C0 + Src1) + (Src0 < C1),
            reference=lambda in0, in1, s0, s1, imm2: (
                in0.astype(np.float32) * s0 + in1
            ) + (in0.astype(np.float32) < s1).astype(np.float32),
        ),
        "ACC_STEP_ANT": Spec(
            body=Src0 + (Src1 < C1),
            reference=lambda in0, in1, s0, s1, imm2: (
                in0.astype(np.float32)
                + (in1.astype(np.float32) < s1).astype(np.float32)
            ),
        ),
    }
    ops = {}
    for name, spec in specs.items():
        if name in D._SUB_OPCODE_FOR_NAME:
            ops[name] = next(op for op in D.OPS if op.name == name)
            continue
        row = max(D._SUB_OPCODE_FOR_NAME.values()) + 1
        assert row < 0x20, "custom-DVE opcode rows exhausted"
        D._SUB_OPCODE_FOR_NAME[name] = row
        shas = {}
        for ver in ("v3", "v4"):
            s = DveOpSpec(
                name=name, opcode=row, uops=lower(spec, ver=ver),
                rd1_en=_has_src1(spec),
            )
            shas[ver] = s.sha(ver)
        op = D.DveOp(name, spec, subdim=False, uops_sha=shas)
        D.OPS.append(op)
        D.CUSTOM_DVE_SPECS[name] = spec
        ops[name] = op
    _CUSTOM_OPS = ops
    return ops


# --------------------------------------------------------------------------
# Program construction
# --------------------------------------------------------------------------

def _build(params, debug=False):
    from concourse import bacc, tile, mybir

    ops = _register_custom_ops()
    lif_op, acc_op = ops["LIF_STEP_ANT"], ops["ACC_STEP_ANT"]

    beta1, th1, beta2, th2, beta3, th3, beta_o, th_o = params
    f32 = mybir.dt.float32
    bf = mybir.dt.bfloat16
    fp8 = mybir.dt.float8e4
    Al = mybir.AluOpType
    AF = mybir.ActivationFunctionType
    DR = mybir.MatmulPerfMode.DoubleRow

    nc = bacc.Bacc(
        "TRN2", target_bir_lowering=False, debug=False, num_devices=NCORES
    )

    xT_d = nc.dram_tensor("xT", [128, 4, 8, NT, BS], fp8, kind="ExternalInput")
    w1_d = nc.dram_tensor("w1T", [128, 16, 4, 2, 128], fp8, kind="ExternalInput")
    w2_d = nc.dram_tensor("w2T", [128, 16, 8, 2, 128], fp8, kind="ExternalInput")
    w3_d = nc.dram_tensor("w3T", [128, 16, 8, 2, 128], fp8, kind="ExternalInput")
    wo_d = nc.dram_tensor("woT", [128, 8, 8, 2, 128], fp8, kind="ExternalInput")
    ball_d = nc.dram_tensor("ball", [128, 56], f32, kind="ExternalInput")
    out_d = nc.dram_tensor("acc_out", [128, 8, BS], bf, kind="ExternalOutput")
    if debug:
        dbg_d = nc.dram_tensor("dbg_s", [128, 3, 2, 16], f32,
                               kind="ExternalOutput")

    with tile.TileContext(nc) as tc:
        with (
            tc.tile_pool(name="const", bufs=1) as cpool,
            tc.tile_pool(name="curp", bufs=3) as curpool,
            tc.tile_pool(name="psp", bufs=8, space="PSUM") as pspool,
        ):
            # x blocked per 16-step chunk: [p, chunk, kt, t_local, b] so each
            # chunk DMA is one contiguous 2KB/partition transfer.
            xT = cpool.tile([128, 4, 8, NT, BS], fp8, tag="xT")
            wt = {
                "w1": cpool.tile([128, 16, 4, 2, 128], fp8, tag="w1", name="w1"),
                "w2": cpool.tile([128, 16, 8, 2, 128], fp8, tag="w2", name="w2"),
                "w3": cpool.tile([128, 16, 8, 2, 128], fp8, tag="w3", name="w3"),
                "wo": cpool.tile([128, 8, 8, 2, 128], fp8, tag="wo", name="wo"),
            }
            ball = cpool.tile([128, 56], f32, tag="ball", name="ball")
            bt = {"b1": ball[:, 0:16], "b2": ball[:, 16:32],
                  "b3": ball[:, 32:48], "bo": ball[:, 48:56]}

            # DMA priority order: the first matmuls need x chunk 0 + the
            # first w1 mt-slices; everything later streams during compute.
            # 2 trigger queues (sync / gpsimd — trigger instructions on the
            # scalar engine measure ~3.3us each and delay the evacuation
            # stream, so it gets none), each processing its triggers in
            # order.
            nc.sync.dma_start(out=wt["w1"][:, 0:1], in_=w1_d[:, 0:1])
            nc.gpsimd.dma_start(out=xT[:, 0], in_=xT_d[:, 0])
            nc.sync.dma_start(out=ball[:], in_=ball_d[:])
            nc.gpsimd.dma_start(out=wt["w1"][:, 1:2], in_=w1_d[:, 1:2])
            nc.sync.dma_start(out=wt["w1"][:, 2:5], in_=w1_d[:, 2:5])
            nc.gpsimd.dma_start(out=wt["w1"][:, 5:8], in_=w1_d[:, 5:8])
            nc.sync.dma_start(out=xT[:, 1], in_=xT_d[:, 1])
            nc.gpsimd.dma_start(out=wt["w1"][:, 8:14], in_=w1_d[:, 8:14])
            nc.sync.dma_start(out=wt["w1"][:, 14:16], in_=w1_d[:, 14:16])
            nc.gpsimd.dma_start(out=xT[:, 2], in_=xT_d[:, 2])
            nc.sync.dma_start(out=xT[:, 3], in_=xT_d[:, 3])
            for q, s0, s1 in ((nc.sync, 0, 4), (nc.gpsimd, 4, 8),
                              (nc.sync, 8, 12), (nc.gpsimd, 12, 16)):
                q.dma_start(out=wt["w2"][:, s0:s1], in_=w2_d[:, s0:s1])
            for q, s0, s1 in ((nc.sync, 0, 4), (nc.gpsimd, 4, 8),
                              (nc.sync, 8, 12), (nc.gpsimd, 12, 16)):
                q.dma_start(out=wt["w3"][:, s0:s1], in_=w3_d[:, s0:s1])
            nc.sync.dma_start(out=wt["wo"][:, 0:4], in_=wo_d[:, 0:4])
            nc.gpsimd.dma_start(out=wt["wo"][:, 4:8], in_=wo_d[:, 4:8])

            # spike tiles per 32-step chunk [p, kt, t_local, b], fp8;
            # sA reused by L3 (L1's spikes are dead once L2 consumed them)
            sA = [cpool.tile([128, 16, 32, BS], fp8, tag=f"sA{c}",
                             name=f"sA{c}") for c in range(2)]
            sB = [cpool.tile([128, 16, 32, BS], fp8, tag=f"sB{c}",
                             name=f"sB{c}") for c in range(2)]

            def gemm_chunk(wtile, btile, KP, MT, nt, rhs_fn, scale):
                """One nt-step column chunk of a layer's matmul.

                rhs_fn(kp) -> [p, 2, nt, BS] fp8 moving AP.
                Returns the SBUF cur tile [128, nt, MT*BS] bf16 (t-major)
                with bias added and the fp8 pre-scale divided out.
                """
                curt = curpool.tile([128, nt, MT * BS], bf, tag="cur")
                for mt in range(MT):
                    ps = pspool.tile([128, nt * BS], f32, tag="ps")
                    for kp in range(KP):
                        nc.tensor.matmul(
                            ps[:],
                            wtile[:, mt, kp],
                            rhs_fn(kp),
                            start=(kp == 0),
                            stop=(kp == KP - 1),
                            perf_mode=DR,
                        )
                    nc.scalar.activation(
                        curt[:, :, mt * BS:(mt + 1) * BS], ps[:], AF.Identity,
                        bias=btile[:, mt:mt + 1], scale=scale,
                    )
                return curt

            def lif_step(mem, nring, t, cur_sl, beta):
                """m̃(t) = beta*m̃(t-1) + c̃(t) + (m̃(t-1) < -1), one DVE op."""
                nc.vector._custom_dve(
                    lif_op, out=mem[:, t % nring],
                    in0=mem[:, (t + nring - 1) % nring],
                    in1=cur_sl, s0=float(beta), s1=-1.0,
                )

            def hidden_chunk_emitter(li, wtile, bname, KP, rhs_src, s_out,
                                     beta, scale, spike_val):
                """Returns emit(t0, nt): emits one chunk's gemm + LIF scan +
                lagged spike flushes for a hidden layer. Chunks must be
                emitted in time order; the final flush goes out with the
                chunk ending at t=T."""
                MT = 16
                mem = cpool.tile([128, 8, MT * BS], bf, tag="mem",
                                 name=f"mem_{li}")
                nc.vector.memset(mem[:, 7], 0.0)

                def flush(t):
                    # flush steps t-3..t of spikes in one strided op; ring
                    # half (t//4)%2, slot index == t%4. spike_val=2 emits
                    # {0,2} spikes: consumers use half-scale weights, so
                    # {0,2}x(W/2) == {0,1}xW exactly and the op stays a
                    # single tensor_scalar ((m < -1) mult 2). Flushes are
                    # emitted one step late so they never read the
                    # immediately-preceding step's output (DVE write-drain
                    # stall).
                    tl = t % 32
                    half = ((t // 4) % 2) * 4
                    out_ap = (
                        s_out[t // 32][:, :, tl - 3:tl + 1, :]
                        .rearrange("p k t b -> p t k b"))
                    if spike_val == 1.0:
                        nc.vector.tensor_scalar(
                            out_ap, mem[:, half:half + 4],
                            -1.0, None, Al.is_lt,
                        )
                    else:
                        nc.vector.tensor_scalar(
                            out_ap, mem[:, half:half + 4],
                            -1.0, spike_val, Al.is_lt, Al.mult,
                        )

                def emit(t0, nt):
                    curt = gemm_chunk(wtile, bt[bname], KP, MT, nt,
                                      lambda kp: rhs_src(kp, t0, nt), scale)
                    for ti in range(nt):
                        t = t0 + ti
                        lif_step(mem, 8, t, curt[:, ti], beta)
                        if t % 4 == 0 and t > 0:
                            flush(t - 1)
                    if t0 + nt == T:
                        flush(T - 1)
                return emit

            def hidden_layer(li, wtile, bname, KP, chunks, rhs_src, s_out,
                             beta, scale, spike_val):
                emit = hidden_chunk_emitter(li, wtile, bname, KP, rhs_src,
                                            s_out, beta, scale, spike_val)
                for t0, nt in chunks:
                    emit(t0, nt)

            def rhs_of_x(kp, t0, nt):
                assert nt == NT and t0 % NT == 0
                return xT[:, t0 // NT, 2 * kp:2 * kp + 2, :, :]

            def rhs_of_s(s):
                def f(kp, t0, nt):
                    c, tl = t0 // 32, t0 % 32
                    return s[c][:, 2 * kp:2 * kp + 2, tl:tl + nt, :]
                return f

            C16 = tuple((16 * i, 16) for i in range(4))
            CL2 = ((0, 32), (32, 16), (48, 16))
            hidden_layer(1, wt["w1"], "b1", 4, C16, rhs_of_x, sA, beta1,
                         1.0 / (WSCALE * XSCALE), 1.0)
            hidden_layer(2, wt["w2"], "b2", 8, CL2, rhs_of_s(sA), sB, beta2,
                         1.0 / WSCALE, 2.0)
            if debug:
                dbg = cpool.tile([128, 3, 2, 16], f32, tag="dbg")
                for c in range(2):
                    nc.vector.tensor_reduce(
                        dbg[:, 0, c, :], sA[c][:], mybir.AxisListType.XY,
                        Al.add,
                    )
                    nc.vector.tensor_reduce(
                        dbg[:, 1, c, :], sB[c][:], mybir.AxisListType.XY,
                        Al.add,
                    )
            # ---- L3 + output layer, woven: Lo chunk gemms/scans are
            # emitted between L3 chunks as soon as the L3 spike flushes
            # they need exist, so the output layer's scan chain overlaps
            # L3's PE stream instead of trailing the whole kernel.
            # Output layer: sA -> 1024 (1000 padded), {0,2} spikes in,
            # halved wo. Ring-4 scan; spike counting is one fused ACC_STEP
            # per step on Vector (acc += (m̃(t) < -1), exact in bf16 since
            # counts are <= 64, lagged one step to dodge the write-drain
            # stall) — no spike tiles, no reduction tail.
            emit_l3 = hidden_chunk_emitter(3, wt["w3"], "b3", 8,
                                           rhs_of_s(sB), sA, beta3,
                                           1.0 / WSCALE, 2.0)
            MT = 8
            memo = cpool.tile([128, 4, MT * BS], bf, tag="memo", name="memo")
            accb = cpool.tile([128, MT * BS], bf, tag="accb", name="accb")
            nc.vector.memset(memo[:, 3], 0.0)
            nc.vector.memset(accb[:], 0.0)

            def acc_step(t):
                nc.vector._custom_dve(
                    acc_op, out=accb[:], in0=accb[:],
                    in1=memo[:, t % 4], s0=0.0, s1=-1.0,
                )

            def emit_lo(t0, nt):
                curt = gemm_chunk(wt["wo"], bt["bo"], 8, MT, nt,
                                  lambda kp: rhs_of_s(sA)(kp, t0, nt),
                                  1.0 / WSCALE)
                for ti in range(nt):
                    t = t0 + ti
                    lif_step(memo, 4, t, curt[:, ti], beta_o)
                    if t > 0:
                        acc_step(t - 1)
                if t0 + nt == T:
                    acc_step(T - 1)

            for t0, nt in ((0, 32), (32, 16), (48, 8), (56, 8)):
                emit_l3(t0, nt)
            for t0, nt in ((0, 16), (16, 16), (32, 16), (48, 8), (56, 4),
                           (60, 4)):
                emit_lo(t0, nt)

            if debug:
                for c in range(2):
                    nc.vector.tensor_reduce(
                        dbg[:, 2, c, :], sA[c][:], mybir.AxisListType.XY,
                        Al.add,
                    )
                nc.gpsimd.dma_start(out=dbg_d[:], in_=dbg[:])

            nc.sync.dma_start(
                out=out_d[:], in_=accb[:].rearrange("p (m b) -> p m b", m=MT),
            )

    nc.compile()
    return nc


def _get_compiled(params, debug=False):
    key = (params, debug)
    if key not in _COMPILED:
        _COMPILED[key] = _build(params, debug=debug)
    return _COMPILED[key]


# --------------------------------------------------------------------------
# Host-side data prep
# --------------------------------------------------------------------------

def _quant_w(w, th, wscale=WSCALE):
    """fp32 [M, K] -> e4m3 with the -wscale/th factor folded in."""
    return np.clip(w * (-wscale / th), -240.0, 240.0).astype(FP8)


def _block_weights(wq, KT, MT):
    """e4m3 [M, K] -> [128, MT, KT//2, 2, 128] with
    out[p, mt, kp, i, f] = wq[mt*128 + f, (2*kp + i)*128 + p]."""
    M, K = wq.shape
    assert M == MT * 128 and K == KT * 128
    return np.ascontiguousarray(
        wq.reshape(MT, 128, KT // 2, 2, 128).transpose(4, 0, 2, 3, 1)
    )


def _prep_inputs(inputs):
    x = np.asarray(inputs["x_seq"], np.float32)

    ths = {k: float(np.asarray(inputs[k], np.float32))
           for k in ("th1", "th2", "th3", "th_out")}
    for k, v in ths.items():
        assert v > 0, f"negated-membrane transform requires {k} > 0, got {v}"

    w1q = _quant_w(np.asarray(inputs["w1"], np.float32), ths["th1"])
    w2q = _quant_w(np.asarray(inputs["w2"], np.float32), ths["th2"])
    # w3 / wo consume {0,2} spikes: quantize at half scale so
    # {0,2} x (W/2) == {0,1} x W exactly.
    w3q = _quant_w(np.asarray(inputs["w3"], np.float32), ths["th3"],
                   WSCALE / 2)
    wo_p = np.zeros((1024, D_H), np.float32)
    wo_p[:D_OUT] = np.asarray(inputs["wo"], np.float32)
    woq = _quant_w(wo_p, ths["th_out"], WSCALE / 2)

    shared = {
        "w1T": _block_weights(w1q, 8, 16),
        "w2T": _block_weights(w2q, 16, 16),
        "w3T": _block_weights(w3q, 16, 16),
        "woT": _block_weights(woq, 16, 8),
    }
    bcols = []
    for b, thk, mt in (
        (inputs["b1"], "th1", 16),
        (inputs["b2"], "th2", 16),
        (inputs["b3"], "th3", 16),
    ):
        col = np.asarray(b, np.float32) * (-1.0 / ths[thk])
        bcols.append(col.reshape(mt, 128).T)
    bo_p = np.zeros(1024, np.float32)
    bo_p[:D_OUT] = np.asarray(inputs["bo"], np.float32) * (-1.0 / ths["th_out"])
    bcols.append(bo_p.reshape(8, 128).T)
    shared["ball"] = np.ascontiguousarray(np.concatenate(bcols, axis=1))

    # per-core x, blocked per 16-step chunk: [p, chunk, kt, t_local, b],
    # fp8 pre-scaled by XSCALE
    xs = []
    xr = np.clip(x * XSCALE, -240.0, 240.0)
    xr = xr.reshape(4, NT, NCORES, BS, 8, 128)  # [c, t, core, b, kt, p]
    for core in range(NCORES):
        xc = xr[:, :, core].transpose(4, 0, 3, 1, 2)  # [p, c, kt, t, b]
        xs.append(np.ascontiguousarray(xc).astype(FP8))
    return shared, xs


def _params_from_inputs(inputs):
    def f(v):
        return float(np.asarray(v, np.float32))
    return (
        float(np.clip(f(inputs["beta1"]), 0.0, 1.0)), f(inputs["th1"]),
        float(np.clip(f(inputs["beta2"]), 0.0, 1.0)), f(inputs["th2"]),
        float(np.clip(f(inputs["beta3"]), 0.0, 1.0)), f(inputs["th3"]),
        float(np.clip(f(inputs["beta_out"]), 0.0, 1.0)), f(inputs["th_out"]),
    )


def _assemble_output(results):
    out = np.zeros((B, D_OUT), np.float32)
    for c in range(NCORES):
        a = np.asarray(results[c]["acc_out"], np.float32)   # [128, 8, 16]
        out[c * BS:(c + 1) * BS] = (
            a.transpose(2, 1, 0).reshape(BS, 1024)[:, :D_OUT]
        )
    return out


# --------------------------------------------------------------------------
# Entry point
# --------------------------------------------------------------------------

def kernel(**inputs):
    from concourse.bass_utils import run_bass_kernel_spmd

    params = _params_from_inputs(inputs)
    debug = bool(int(os.environ.get("SNN_KERNEL_DEBUG", "0")))
    nc = _get_compiled(params, debug=debug)
    shared, xs = _prep_inputs(inputs)
    in_maps = [dict(shared, xT=xs[c]) for c in range(NCORES)]
    trace = bool(int(os.environ.get("SNN_KERNEL_TRACE", "0")))
    try:
        res = run_bass_kernel_spmd(
            nc, in_maps, list(range(NCORES)), trace=trace
        )
    except ModuleNotFoundError:
        res = run_bass_kernel_spmd(nc, in_maps, list(range(NCORES)))
    out = _assemble_output(res.results)
    kernel.last_results = res
    return out


# revision 32
# speedup vs baseline: 1.1968x; 1.1968x over previous
"""Trainium2 Bass kernel for the DenseSNN problem (4-layer LIF spiking MLP).

Strategy
--------
Data-parallel over batch: B=128 is split into 8 shards of 16, one per
NeuronCore, with weights replicated (no collectives at all).

Per core the time recurrence is restructured layer-at-a-time: layer l's
input spikes for ALL timesteps are known once layer l-1's LIF scan
finishes, so each layer becomes a sequence of batched matmuls over
(t, b) column chunks followed by a sequential 64-step elementwise LIF
scan on the Vector engine, run on the negated membrane m̃ = -mem/th (the
-1/th is folded into weights/bias host-side).

The LIF step is ONE custom DVE instruction (registered at import):

    m̃(t) = beta*m̃(t-1) + c̃(t) + (m̃(t-1) < -1)
    spk(t) = (m̃(t) < -1)                        (flushed 4 steps at a time)

All matmul operands are fp8 e4m3 in DoubleRow perf mode (K=256 per
instruction — 157 TF/s, 2x the bf16 rate). Spikes are exactly
representable in fp8 (0.0/1.0); weights are pre-scaled by 2^12
host-side so their magnitudes sit in e4m3's normal range, and the scale
is divided back out (exact power of two) during the PSUM->SBUF
evacuation on the Scalar engine, which also adds the bias.

Schedule: 16-step column chunks everywhere, strict layer-sequential PE
order (each layer's PE time far exceeds the scan lag, so the PE never
stalls after the initial DMA), weight/x DMAs issued in priority order
across 4 engine queues (x chunk0 + w1 first) with per-slice gating so
the first matmul starts as soon as ~0.75MB has landed. The output layer
writes its full membrane history; spikes are extracted chunk-wise and
accumulated with one tensor_reduce at the end.
"""

import os
import sys

import numpy as np
import ml_dtypes

if "/opt/trn_rl_repo" not in sys.path:
    sys.path.insert(0, "/opt/trn_rl_repo")

T, B, D_IN, D_H, D_OUT = 64, 128, 1024, 2048, 1000
NCORES = 8
BS = B // NCORES           # 16 batch rows per core
COLS = T * BS              # 1024 (t, b) columns
NT = 16                    # timesteps per column chunk
NCH = T // NT              # 4 chunks per layer

WSCALE = 4096.0            # weight pre-scale into e4m3 normal range
XSCALE = 16.0              # x pre-scale

BF16 = ml_dtypes.bfloat16
FP8 = ml_dtypes.float8_e4m3

_COMPILED = {}
_CUSTOM_OPS = None


def _register_custom_ops():
    """Register two fused custom DVE ops (each lowers to a single uop, so
    they run at native 1-instruction DVE throughput):

      LIF_STEP_ANT: out = in0*s0 + in1 + (in0 < s1)
        i.e. m̃(t) = beta*m̃(t-1) + c̃(t) + spike(t-1), with the reset term
        recomputed from the previous membrane.

      ACC_STEP_ANT: out = in0 + (in1 < s1)
        i.e. acc += spike(t) for the output layer's spike counting —
        no spike materialization, no reduction tail.
    """
    global _CUSTOM_OPS
    if _CUSTOM_OPS is not None:
        return _CUSTOM_OPS
    from concourse.dve_spec import Spec, Src0, Src1, C0, C1, lower, _has_src1
    from concourse.dve_uop import DveOpSpec
    from concourse import dve_ops as D

    specs = {
        "LIF_STEP_ANT": Spec(
            body=(Src0 * C0 + Src1) + (Src0 < C1),
            reference=lambda in0, in1, s0, s1, imm2: (
                in0.astype(np.float32) * s0 + in1
            ) + (in0.astype(np.float32) < s1).astype(np.float32),
        ),
        "ACC_STEP_ANT": Spec(
            body=Src0 + (Src1 < C1),
            reference=lambda in0, in1, s0, s1, imm2: (
                in0.astype(np.float32)
                + (in1.astype(np.float32) < s1).astype(np.float32)
            ),
        ),
    }
    ops = {}
    for name, spec in specs.items():
        if name in D._SUB_OPCODE_FOR_NAME:
            ops[name] = next(op for op in D.OPS if op.name == name)
            continue
        row = max(D._SUB_OPCODE_FOR_NAME.values()) + 1
        assert row < 0x20, "custom-DVE opcode rows exhausted"
        D._SUB_OPCODE_FOR_NAME[name] = row
        shas = {}
        for ver in ("v3", "v4"):
            s = DveOpSpec(
                name=name, opcode=row, uops=lower(spec, ver=ver),
                rd1_en=_has_src1(spec),
            )
            shas[ver] = s.sha(ver)
        op = D.DveOp(name, spec, subdim=False, uops_sha=shas)
        D.OPS.append(op)
        D.CUSTOM_DVE_SPECS[name] = spec
        ops[name] = op
    _CUSTOM_OPS = ops
    return ops


# --------------------------------------------------------------------------
# Program construction
# --------------------------------------------------------------------------

def _build(params, debug=False):
    from concourse import bacc, tile, mybir

    ops = _register_custom_ops()
    lif_op, acc_op = ops["LIF_STEP_ANT"], ops["ACC_STEP_ANT"]

    beta1, th1, beta2, th2, beta3, th3, beta_o, th_o = params
    f32 = mybir.dt.float32
    bf = mybir.dt.bfloat16
    fp8 = mybir.dt.float8e4
    Al = mybir.AluOpType
    AF = mybir.ActivationFunctionType
    DR = mybir.MatmulPerfMode.DoubleRow

    nc = bacc.Bacc(
        "TRN2", target_bir_lowering=False, debug=False, num_devices=NCORES
    )

    xT_d = nc.dram_tensor("xT", [128, 4, 8, NT, BS], fp8, kind="ExternalInput")
    w1_d = nc.dram_tensor("w1T", [128, 16, 4, 2, 128], fp8, kind="ExternalInput")
    w2_d = nc.dram_tensor("w2T", [128, 16, 8, 2, 128], fp8, kind="ExternalInput")
    w3_d = nc.dram_tensor("w3T", [128, 16, 8, 2, 128], fp8, kind="ExternalInput")
    wo_d = nc.dram_tensor("woT", [128, 8, 8, 2, 128], fp8, kind="ExternalInput")
    ball_d = nc.dram_tensor("ball", [128, 56], f32, kind="ExternalInput")
    out_d = nc.dram_tensor("acc_out", [128, 8, BS], bf, kind="ExternalOutput")
    if debug:
        dbg_d = nc.dram_tensor("dbg_s", [128, 3, 2, 16], f32,
                               kind="ExternalOutput")

    with tile.TileContext(nc) as tc:
        with (
            tc.tile_pool(name="const", bufs=1) as cpool,
            tc.tile_pool(name="curp", bufs=3) as curpool,
            tc.tile_pool(name="psp", bufs=8, space="PSUM") as pspool,
        ):
            # x blocked per 16-step chunk: [p, chunk, kt, t_local, b] so each
            # chunk DMA is one contiguous 2KB/partition transfer.
            xT = cpool.tile([128, 4, 8, NT, BS], fp8, tag="xT")
            wt = {
                "w1": cpool.tile([128, 16, 4, 2, 128], fp8, tag="w1", name="w1"),
                "w2": cpool.tile([128, 16, 8, 2, 128], fp8, tag="w2", name="w2"),
                "w3": cpool.tile([128, 16, 8, 2, 128], fp8, tag="w3", name="w3"),
                "wo": cpool.tile([128, 8, 8, 2, 128], fp8, tag="wo", name="wo"),
            }
            ball = cpool.tile([128, 56], f32, tag="ball", name="ball")
            bt = {"b1": ball[:, 0:16], "b2": ball[:, 16:32],
                  "b3": ball[:, 32:48], "bo": ball[:, 48:56]}

            # DMA priority order: the first matmuls need x chunk 0 + the
            # first w1 mt-slices; everything later streams during compute.
            # 2 trigger queues (sync / gpsimd — trigger instructions on the
            # scalar engine measure ~3.3us each and delay the evacuation
            # stream, so it gets none), each processing its triggers in
            # order.
            nc.sync.dma_start(out=ball[:], in_=ball_d[:])
            nc.gpsimd.dma_start(out=xT[:, 0], in_=xT_d[:, 0])
            nc.sync.dma_start(out=wt["w1"][:, 0:1], in_=w1_d[:, 0:1])
            nc.gpsimd.dma_start(out=wt["w1"][:, 1:2], in_=w1_d[:, 1:2])
            nc.sync.dma_start(out=wt["w1"][:, 2:5], in_=w1_d[:, 2:5])
            nc.gpsimd.dma_start(out=wt["w1"][:, 5:8], in_=w1_d[:, 5:8])
            nc.sync.dma_start(out=xT[:, 1], in_=xT_d[:, 1])
            nc.gpsimd.dma_start(out=wt["w1"][:, 8:14], in_=w1_d[:, 8:14])
            nc.sync.dma_start(out=wt["w1"][:, 14:16], in_=w1_d[:, 14:16])
            nc.gpsimd.dma_start(out=xT[:, 2], in_=xT_d[:, 2])
            nc.sync.dma_start(out=xT[:, 3], in_=xT_d[:, 3])
            for q, s0, s1 in ((nc.sync, 0, 4), (nc.gpsimd, 4, 8),
                              (nc.sync, 8, 12), (nc.gpsimd, 12, 16)):
                q.dma_start(out=wt["w2"][:, s0:s1], in_=w2_d[:, s0:s1])
            for q, s0, s1 in ((nc.sync, 0, 4), (nc.gpsimd, 4, 8),
                              (nc.sync, 8, 12), (nc.gpsimd, 12, 16)):
                q.dma_start(out=wt["w3"][:, s0:s1], in_=w3_d[:, s0:s1])
            nc.sync.dma_start(out=wt["wo"][:, 0:4], in_=wo_d[:, 0:4])
            nc.gpsimd.dma_start(out=wt["wo"][:, 4:8], in_=wo_d[:, 4:8])

            # spike tiles per 32-step chunk [p, kt, t_local, b], fp8;
            # sA reused by L3 (L1's spikes are dead once L2 consumed them)
            sA = [cpool.tile([128, 16, 32, BS], fp8, tag=f"sA{c}",
                             name=f"sA{c}") for c in range(2)]
            sB = [cpool.tile([128, 16, 32, BS], fp8, tag=f"sB{c}",
                             name=f"sB{c}") for c in range(2)]

            def gemm_chunk(wtile, btile, KP, MT, nt, rhs_fn, scale):
                """One nt-step column chunk of a layer's matmul.

                rhs_fn(kp) -> [p, 2, nt, BS] fp8 moving AP.
                Returns the SBUF cur tile [128, nt, MT*BS] bf16 (t-major)
                with bias added and the fp8 pre-scale divided out.
                """
                curt = curpool.tile([128, nt, MT * BS], bf, tag="cur")
                for mt in range(MT):
                    ps = pspool.tile([128, nt * BS], f32, tag="ps")
                    for kp in range(KP):
                        nc.tensor.matmul(
                            ps[:],
                            wtile[:, mt, kp],
                            rhs_fn(kp),
                            start=(kp == 0),
                            stop=(kp == KP - 1),
                            perf_mode=DR,
                        )
                    nc.scalar.activation(
                        curt[:, :, mt * BS:(mt + 1) * BS], ps[:], AF.Identity,
                        bias=btile[:, mt:mt + 1], scale=scale,
                    )
                return curt

            def lif_step(mem, nring, t, cur_sl, beta):
                """m̃(t) = beta*m̃(t-1) + c̃(t) + (m̃(t-1) < -1), one DVE op."""
                nc.vector._custom_dve(
                    lif_op, out=mem[:, t % nring],
                    in0=mem[:, (t + nring - 1) % nring],
                    in1=cur_sl, s0=float(beta), s1=-1.0,
                )

            def hidden_chunk_emitter(li, wtile, bname, KP, rhs_src, s_out,
                                     beta, scale, spike_val):
                """Returns emit(t0, nt): emits one chunk's gemm + LIF scan +
                lagged spike flushes for a hidden layer. Chunks must be
                emitted in time order; the final flush goes out with the
                chunk ending at t=T."""
                MT = 16
                mem = cpool.tile([128, 8, MT * BS], bf, tag="mem",
                                 name=f"mem_{li}")
                nc.vector.memset(mem[:, 7], 0.0)

                def flush(t):
                    # flush steps t-3..t of spikes in one strided op; ring
                    # half (t//4)%2, slot index == t%4. spike_val=2 emits
                    # {0,2} spikes: consumers use half-scale weights, so
                    # {0,2}x(W/2) == {0,1}xW exactly and the op stays a
                    # single tensor_scalar ((m < -1) mult 2). Flushes are
                    # emitted one step late so they never read the
                    # immediately-preceding step's output (DVE write-drain
                    # stall).
                    tl = t % 32
                    half = ((t // 4) % 2) * 4
                    out_ap = (
                        s_out[t // 32][:, :, tl - 3:tl + 1, :]
                        .rearrange("p k t b -> p t k b"))
                    if spike_val == 1.0:
                        nc.vector.tensor_scalar(
                            out_ap, mem[:, half:half + 4],
                            -1.0, None, Al.is_lt,
                        )
                    else:
                        nc.vector.tensor_scalar(
                            out_ap, mem[:, half:half + 4],
                            -1.0, spike_val, Al.is_lt, Al.mult,
                        )

                def emit(t0, nt):
                    curt = gemm_chunk(wtile, bt[bname], KP, MT, nt,
                                      lambda kp: rhs_src(kp, t0, nt), scale)
                    for ti in range(nt):
                        t = t0 + ti
                        lif_step(mem, 8, t, curt[:, ti], beta)
                        if t % 4 == 0 and t > 0:
                            flush(t - 1)
                    if t0 + nt == T:
                        flush(T - 1)
                return emit

            def hidden_layer(li, wtile, bname, KP, chunks, rhs_src, s_out,
                             beta, scale, spike_val):
                emit = hidden_chunk_emitter(li, wtile, bname, KP, rhs_src,
                                            s_out, beta, scale, spike_val)
                for t0, nt in chunks:
                    emit(t0, nt)

            def rhs_of_x(kp, t0, nt):
                assert nt == NT and t0 % NT == 0
                return xT[:, t0 // NT, 2 * kp:2 * kp + 2, :, :]

            def rhs_of_s(s):
                def f(kp, t0, nt):
                    c, tl = t0 // 32, t0 % 32
                    return s[c][:, 2 * kp:2 * kp + 2, tl:tl + nt, :]
                return f

            C16 = tuple((16 * i, 16) for i in range(4))
            CL2 = ((0, 32), (32, 16), (48, 16))
            hidden_layer(1, wt["w1"], "b1", 4, C16, rhs_of_x, sA, beta1,
                         1.0 / (WSCALE * XSCALE), 1.0)
            hidden_layer(2, wt["w2"], "b2", 8, CL2, rhs_of_s(sA), sB, beta2,
                         1.0 / WSCALE, 2.0)
            if debug:
                dbg = cpool.tile([128, 3, 2, 16], f32, tag="dbg")
                for c in range(2):
                    nc.vector.tensor_reduce(
                        dbg[:, 0, c, :], sA[c][:], mybir.AxisListType.XY,
                        Al.add,
                    )
                    nc.vector.tensor_reduce(
                        dbg[:, 1, c, :], sB[c][:], mybir.AxisListType.XY,
                        Al.add,
                    )
            # ---- L3 + output layer, woven: Lo chunk gemms/scans are
            # emitted between L3 chunks as soon as the L3 spike flushes
            # they need exist, so the output layer's scan chain overlaps
            # L3's PE stream instead of trailing the whole kernel.
            # Output layer: sA -> 1024 (1000 padded), {0,2} spikes in,
            # halved wo. Ring-4 scan; spike counting is one fused ACC_STEP
            # per step on Vector (acc += (m̃(t) < -1), exact in bf16 since
            # counts are <= 64, lagged one step to dodge the write-drain
            # stall) — no spike tiles, no reduction tail.
            emit_l3 = hidden_chunk_emitter(3, wt["w3"], "b3", 8,
                                           rhs_of_s(sB), sA, beta3,
                                           1.0 / WSCALE, 2.0)
            MT = 8
            memo = cpool.tile([128, 4, MT * BS], bf, tag="memo", name="memo")
            accb = cpool.tile([128, MT * BS], bf, tag="accb", name="accb")
            nc.vector.memset(memo[:, 3], 0.0)
            nc.vector.memset(accb[:], 0.0)

            def acc_step(t):
                nc.vector._custom_dve(
                    acc_op, out=accb[:], in0=accb[:],
                    in1=memo[:, t % 4], s0=0.0, s1=-1.0,
                )

            def emit_lo(t0, nt):
                curt = gemm_chunk(wt["wo"], bt["bo"], 8, MT, nt,
                                  lambda kp: rhs_of_s(sA)(kp, t0, nt),
                                  1.0 / WSCALE)
                for ti in range(nt):
                    t = t0 + ti
                    lif_step(memo, 4, t, curt[:, ti], beta_o)
                    if t > 0:
                        acc_step(t - 1)
                if t0 + nt == T:
                    acc_step(T - 1)

            for t0, nt in ((0, 32), (32, 16), (48, 8), (56, 8)):
                emit_l3(t0, nt)
            for t0, nt in ((0, 16), (16, 16), (32, 16), (48, 8), (56, 4),
                           (60, 4)):
                emit_lo(t0, nt)

            if debug:
                for c in range(2):
                    nc.vector.tensor_reduce(
                        dbg[:, 2, c, :], sA[c][:], mybir.AxisListType.XY,
                        Al.add,
                    )
                nc.gpsimd.dma_start(out=dbg_d[:], in_=dbg[:])

            nc.sync.dma_start(
                out=out_d[:], in_=accb[:].rearrange("p (m b) -> p m b", m=MT),
            )

    nc.compile()
    return nc


def _get_compiled(params, debug=False):
    key = (params, debug)
    if key not in _COMPILED:
        _COMPILED[key] = _build(params, debug=debug)
    return _COMPILED[key]


# --------------------------------------------------------------------------
# Host-side data prep
# --------------------------------------------------------------------------

def _quant_w(w, th, wscale=WSCALE):
    """fp32 [M, K] -> e4m3 with the -wscale/th factor folded in."""
    return np.clip(w * (-wscale / th), -240.0, 240.0).astype(FP8)


def _block_weights(wq, KT, MT):
    """e4m3 [M, K] -> [128, MT, KT//2, 2, 128] with
    out[p, mt, kp, i, f] = wq[mt*128 + f, (2*kp + i)*128 + p]."""
    M, K = wq.shape
    assert M == MT * 128 and K == KT * 128
    return np.ascontiguousarray(
        wq.reshape(MT, 128, KT // 2, 2, 128).transpose(4, 0, 2, 3, 1)
    )


def _prep_inputs(inputs):
    x = np.asarray(inputs["x_seq"], np.float32)

    ths = {k: float(np.asarray(inputs[k], np.float32))
           for k in ("th1", "th2", "th3", "th_out")}
    for k, v in ths.items():
        assert v > 0, f"negated-membrane transform requires {k} > 0, got {v}"

    w1q = _quant_w(np.asarray(inputs["w1"], np.float32), ths["th1"])
    w2q = _quant_w(np.asarray(inputs["w2"], np.float32), ths["th2"])
    # w3 / wo consume {0,2} spikes: quantize at half scale so
    # {0,2} x (W/2) == {0,1} x W exactly.
    w3q = _quant_w(np.asarray(inputs["w3"], np.float32), ths["th3"],
                   WSCALE / 2)
    wo_p = np.zeros((1024, D_H), np.float32)
    wo_p[:D_OUT] = np.asarray(inputs["wo"], np.float32)
    woq = _quant_w(wo_p, ths["th_out"], WSCALE / 2)

    shared = {
        "w1T": _block_weights(w1q, 8, 16),
        "w2T": _block_weights(w2q, 16, 16),
        "w3T": _block_weights(w3q, 16, 16),
        "woT": _block_weights(woq, 16, 8),
    }
    bcols = []
    for b, thk, mt in (
        (inputs["b1"], "th1", 16),
        (inputs["b2"], "th2", 16),
        (inputs["b3"], "th3", 16),
    ):
        col = np.asarray(b, np.float32) * (-1.0 / ths[thk])
        bcols.append(col.reshape(mt, 128).T)
    bo_p = np.zeros(1024, np.float32)
    bo_p[:D_OUT] = np.asarray(inputs["bo"], np.float32) * (-1.0 / ths["th_out"])
    bcols.append(bo_p.reshape(8, 128).T)
    shared["ball"] = np.ascontiguousarray(np.concatenate(bcols, axis=1))

    # per-core x, blocked per 16-step chunk: [p, chunk, kt, t_local, b],
    # fp8 pre-scaled by XSCALE
    xs = []
    xr = np.clip(x * XSCALE, -240.0, 240.0)
    xr = xr.reshape(4, NT, NCORES, BS, 8, 128)  # [c, t, core, b, kt, p]
    for core in range(NCORES):
        xc = xr[:, :, core].transpose(4, 0, 3, 1, 2)  # [p, c, kt, t, b]
        xs.append(np.ascontiguousarray(xc).astype(FP8))
    return shared, xs


def _params_from_inputs(inputs):
    def f(v):
        return float(np.asarray(v, np.float32))
    return (
        float(np.clip(f(inputs["beta1"]), 0.0, 1.0)), f(inputs["th1"]),
        float(np.clip(f(inputs["beta2"]), 0.0, 1.0)), f(inputs["th2"]),
        float(np.clip(f(inputs["beta3"]), 0.0, 1.0)), f(inputs["th3"]),
        float(np.clip(f(inputs["beta_out"]), 0.0, 1.0)), f(inputs["th_out"]),
    )


def _assemble_output(results):
    out = np.zeros((B, D_OUT), np.float32)
    for c in range(NCORES):
        a = np.asarray(results[c]["acc_out"], np.float32)   # [128, 8, 16]
        out[c * BS:(c + 1) * BS] = (
            a.transpose(2, 1, 0).reshape(BS, 1024)[:, :D_OUT]
        )
    return out


# --------------------------------------------------------------------------
# Entry point
# --------------------------------------------------------------------------

def kernel(**inputs):
    from concourse.bass_utils import run_bass_kernel_spmd

    params = _params_from_inputs(inputs)
    debug = bool(int(os.environ.get("SNN_KERNEL_DEBUG", "0")))
    nc = _get_compiled(params, debug=debug)
    shared, xs = _prep_inputs(inputs)
    in_maps = [dict(shared, xT=xs[c]) for c in range(NCORES)]
    trace = bool(int(os.environ.get("SNN_KERNEL_TRACE", "0")))
    try:
        res = run_bass_kernel_spmd(
            nc, in_maps, list(range(NCORES)), trace=trace
        )
    except ModuleNotFoundError:
        res = run_bass_kernel_spmd(nc, in_maps, list(range(NCORES)))
    out = _assemble_output(res.results)
    kernel.last_results = res
    return out
